# revision 1
# baseline (speedup 1.0000x reference)
"""Trainium2 Bass kernel for nn_ContextLabel (GNN label propagation).

Computation: 10 iterations of Y = masked(adj @ Y) on [10000,16], then
straight-through gumbel one-hot, dist = (adj!=0) @ Yh row-normalized,
output mean((dist - pseudo_labels)^2)  (scalar).

Strategy (8 NeuronCores, row-parallel):
 - core c owns rows [1250c, 1250c+1250)
 - adj^T shard (fp8 e4m3, [10000 x 1250]) stays RESIDENT in SBUF; all 10
   propagation passes stream it from SBUF through the tensor engine with
   Y (fp16) as the stationary operand: out^T[16,1250] = Y^T @ adjT.
 - per-iteration AllGather of the fp16 Y slice across the 8 cores.
 - final pass streams the 0/1 mask (fp8) from HBM, Yh (fp8 one-hot)
   stationary; row-normalize + squared-error partials on device.
fp8 adj values give ~1 argmax flip out of 10000 rows (verified on host:
final relerr ~3e-5); Y in fp16 is bit-exact vs fp32 for the argmax.
"""

import hashlib
import os
import shutil
import sys
from pathlib import Path

import numpy as np
import ml_dtypes

sys.path.insert(0, "/opt/trn_rl_repo")

import concourse.bass as bass  # noqa: E402
import concourse.mybir as mybir  # noqa: E402
import concourse.tile as tile  # noqa: E402
from concourse import bacc  # noqa: E402
import concourse.bass2jax as bass2jax  # noqa: E402
from concourse.bass_utils import run_bass_kernel_spmd  # noqa: E402
from concourse.masks import make_identity  # noqa: E402

F8 = ml_dtypes.float8_e4m3
NCORES = 8
N = 10000
C = 16
R = N // NCORES           # 1250 rows per core
NBLK = R // 128           # 10 blocks of local rows (1250 = 9*128 + 82 -> no!)
# careful: 1250 = 9*128 + 82; use 10 blocks of 125? No - we transpose in
# [16,128] slices; 1250 splits as 9 full 128-slices + one 82-slice.
FULLB = R // 128          # 9 full 128-col blocks
TAILB = R - FULLB * 128   # 82
KCH = 78                  # full 128-row contraction chunks
KTAIL = N - KCH * 128     # 16
NCHT = KCH + 1            # 79 chunk slots in tiled [128, 79*16] layout
SL = [(0, 512), (512, 512), (1024, 226)]  # psum bank slices of 1250
MGRP = 8                  # mask-stream chunks per DMA group

_NEFF_CACHE = Path.home() / ".cache" / "bass_neff"


def _install_neff_cache():
    orig = bass2jax.compile_bir_kernel
    if getattr(bass2jax.compile_bir_kernel, "_cached", False):
        return

    def cached(bir_json, tmpdir, neff_name="file.neff"):
        h = hashlib.sha256(bir_json).hexdigest()
        p = _NEFF_CACHE / f"{h}.neff"
        dst = os.path.join(tmpdir, neff_name)
        if p.exists():
            shutil.copy(p, dst)
            return dst
        out = orig(bir_json, tmpdir, neff_name)
        try:
            _NEFF_CACHE.mkdir(parents=True, exist_ok=True)
            shutil.copy(out, p)
        except OSError:
            pass
        return out

    cached._cached = True
    bass2jax.compile_bir_kernel = cached


def build_program():
    nc = bacc.Bacc(
        "TRN2", target_bir_lowering=False, debug=False,
        enable_asserts=False, num_devices=NCORES,
    )
    f8, f16, f32 = mybir.dt.float8e4, mybir.dt.float16, mybir.dt.float32

    adjT_d = nc.dram_tensor("adjT8", [N, R], f8, kind="ExternalInput")
    maskT_d = nc.dram_tensor("maskT8", [N, R], f8, kind="ExternalInput")
    gumt_d = nc.dram_tensor("gumt", [128, NCHT * C], f32, kind="ExternalInput")
    labmt_d = nc.dram_tensor("labmt", [128, NCHT * C], f16, kind="ExternalInput")
    m16t_d = nc.dram_tensor("m16t", [128, NCHT * C], mybir.dt.uint8, kind="ExternalInput")
    mT16_d = nc.dram_tensor("mT16", [C, R], mybir.dt.uint8, kind="ExternalInput")
    labT_d = nc.dram_tensor("labT", [C, R], f16, kind="ExternalInput")
    pst_d = nc.dram_tensor("pst", [128, FULLB + 1, C], f32, kind="ExternalInput")
    out_d = nc.dram_tensor("out_sq", [128, FULLB + 1], f32, kind="ExternalOutput")

    with tile.TileContext(nc) as tc:
        with (
            tc.tile_pool(name="sb", bufs=1) as sb,
            tc.tile_pool(name="mtp", bufs=2) as mtp,
            tc.tile_pool(name="ps", bufs=2, space="PSUM") as ps,
            tc.tile_pool(name="dram", bufs=2, space="DRAM") as dram,
        ):
            # ---- resident tiles -------------------------------------
            at_g = []
            for g in range(10):
                kc = 8 if g < 9 else 6
                t = sb.tile([128, kc * R], f8, name=f"at{g}", tag=f"at{g}")
                at_g.append(t)
            at_last = sb.tile([KTAIL, R], f8)
            ycur = sb.tile([128, NCHT * C], f16)
            gumt = sb.tile([128, NCHT * C], f32)
            labmt = sb.tile([128, NCHT * C], f16)
            m16t = sb.tile([128, NCHT * C], mybir.dt.uint8)
            mT16 = sb.tile([C, R], mybir.dt.uint8)
            labT = sb.tile([C, R], f16)
            pst = sb.tile([128, FULLB + 1, C], f32)
            ident = sb.tile([C, C], f16)
            yT = sb.tile([C, R], f16)
            yloc = sb.tile([128, FULLB + 1, C], f16)
            logits = sb.tile([128, NCHT, C], f32)
            rmax = sb.tile([128, NCHT], f32)
            yh16 = sb.tile([128, NCHT * C], f16)
            yh8 = sb.tile([128, NCHT * C], f8)

            make_identity(nc, ident[:])

            # ---- initial loads --------------------------------------
            for g in range(10):
                kc = 8 if g < 9 else 6
                src = adjT_d[g * 8 * 128:(g * 8 + kc) * 128, :]
                nc.sync.dma_start(
                    out=at_g[g][:].rearrange("p (k i) -> p k i", k=kc),
                    in_=src.rearrange("(k p) i -> p k i", p=128),
                )
            nc.sync.dma_start(out=at_last[:], in_=adjT_d[KCH * 128:N, :])
            nc.sync.dma_start(out=ycur[:], in_=labmt_d[:])  # Y0 = labels*m
            nc.sync.dma_start(out=gumt[:], in_=gumt_d[:])
            nc.sync.dma_start(out=labmt[:], in_=labmt_d[:])
            nc.sync.dma_start(out=m16t[:], in_=m16t_d[:])
            nc.sync.dma_start(out=mT16[:], in_=mT16_d[:])
            nc.sync.dma_start(out=labT[:], in_=labT_d[:])
            nc.sync.dma_start(out=pst[:], in_=pst_d[:])

            def mm_pass(acc, lhs_tile, rhs_chunk):
                """acc[16,1250] += lhs.T @ rhsT  over all 79 chunks."""
                for k in range(NCHT):
                    if k < KCH:
                        lhsT = lhs_tile[:, k * C:(k + 1) * C]
                        rhs_t, rhs_off = rhs_chunk(k)
                    else:
                        lhsT = lhs_tile[0:KTAIL, k * C:(k + 1) * C]
                        rhs_t, rhs_off = rhs_chunk(k)
                    for (s0, sw) in SL:
                        nc.tensor.matmul(
                            acc[:, s0:s0 + sw],
                            lhsT,
                            rhs_t[0:lhsT.partition_size(),
                                  rhs_off + s0:rhs_off + s0 + sw],
                            start=(k == 0), stop=(k == NCHT - 1),
                        )

            def at_chunk(k):
                if k < KCH:
                    g, j = k // 8, k % 8
                    return at_g[g], j * R
                return at_last, 0

            # ---- 10 propagation iterations --------------------------
            for t in range(10):
                acc = ps.tile([C, R], f32, name=f"acc{t}", tag="acc")
                mm_pass(acc, ycur, at_chunk)
                # masked overwrite in transposed layout, cast to fp16
                nc.vector.tensor_copy(yT[:], acc[:])
                nc.vector.copy_predicated(yT[:], mT16[:], labT[:])
                # transpose [16,1250] -> [1250,16] via PE, in 128-col blocks
                trp = ps.tile([128, (FULLB + 1) * C], f16, name=f"trp{t}", tag="trp")
                for b in range(FULLB):
                    nc.tensor.transpose(
                        trp[:, b * C:(b + 1) * C],
                        yT[:, b * 128:(b + 1) * 128], ident[:],
                    )
                nc.tensor.transpose(
                    trp[0:TAILB, FULLB * C:(FULLB + 1) * C],
                    yT[:, FULLB * 128:R], ident[:],
                )
                nc.vector.tensor_copy(yloc[:].rearrange("p b c -> p (b c)"), trp[:])
                # exchange local rows -> full Y
                cc_in = dram.tile([R, C], f16, name=f"ccin{t}", tag="ccin")
                cc_out = dram.tile([N, C], f16, name=f"ccout{t}", tag="ccout",
                                   addr_space="Shared")
                nc.sync.dma_start(
                    out=cc_in[0:FULLB * 128, :].rearrange("(b p) c -> p b c", p=128),
                    in_=yloc[0:128, 0:FULLB, :],
                )
                nc.sync.dma_start(
                    out=cc_in[FULLB * 128:R, :],
                    in_=yloc[0:TAILB, FULLB, :],
                )
                nc.gpsimd.collective_compute(
                    "AllGather", mybir.AluOpType.bypass,
                    replica_groups=[list(range(NCORES))],
                    ins=[cc_in[:]], outs=[cc_out[:]],
                )
                nc.sync.dma_start(
                    out=ycur[:, 0:KCH * C].rearrange("p (k c) -> p k c", c=C),
                    in_=cc_out[0:KCH * 128, :].rearrange("(k p) c -> p k c", p=128),
                )
                nc.sync.dma_start(
                    out=ycur[0:KTAIL, KCH * C:NCHT * C],
                    in_=cc_out[KCH * 128:N, :],
                )

            # ---- straight-through gumbel one-hot --------------------
            nc.vector.tensor_tensor(
                logits[:].rearrange("p k c -> p (k c)"), ycur[:], gumt[:],
                mybir.AluOpType.add,
            )
            nc.vector.tensor_reduce(
                rmax[:], logits[:], axis=mybir.AxisListType.X,
                op=mybir.AluOpType.max,
            )
            nc.vector.tensor_tensor(
                yh16[:].rearrange("p (k c) -> p k c", c=C),
                logits[:],
                rmax[:].unsqueeze(2).broadcast_to([128, NCHT, C]),
                mybir.AluOpType.is_equal,
            )
            nc.vector.copy_predicated(yh16[:], m16t[:], labmt[:])
            nc.vector.tensor_copy(yh8[:], yh16[:])

            # ---- final pass: dist^T = Yh^T @ maskT ------------------
            mt_tiles = {}
            for g in range(10):
                kc = 8 if g < 9 else 6
                mt = mtp.tile([128, kc * R], f8, name=f"mt{g}", tag="mt")
                src = maskT_d[g * 8 * 128:(g * 8 + kc) * 128, :]
                nc.sync.dma_start(
                    out=mt[:].rearrange("p (k i) -> p k i", k=kc),
                    in_=src.rearrange("(k p) i -> p k i", p=128),
                )
                mt_tiles[g] = mt
            mt_last = sb.tile([KTAIL, R], f8)
            nc.sync.dma_start(out=mt_last[:], in_=maskT_d[KCH * 128:N, :])

            def mt_chunk(k):
                if k < KCH:
                    g, j = k // 8, k % 8
                    return mt_tiles[g], j * R
                return mt_last, 0

            dacc = ps.tile([C, R], f32, tag="acc")
            mm_pass(dacc, yh8, mt_chunk)

            # ---- normalize + squared error --------------------------
            dT = sb.tile([C, R], f32)
            nc.vector.tensor_copy(dT[:], dacc[:])
            ident32 = sb.tile([C, C], f32)
            make_identity(nc, ident32[:])
            trd = ps.tile([128, (FULLB + 1) * C], f32, tag="trp")
            # tail block: partitions >= TAILB are never written by the
            # transpose; zero them so normalization stays finite
            nc.vector.memset(trd[:, FULLB * C:(FULLB + 1) * C], 0.0)
            for b in range(FULLB):
                nc.tensor.transpose(
                    trd[:, b * C:(b + 1) * C],
                    dT[:, b * 128:(b + 1) * 128], ident32[:],
                )
            nc.tensor.transpose(
                trd[0:TAILB, FULLB * C:(FULLB + 1) * C],
                dT[:, FULLB * 128:R], ident32[:],
            )
            dist = sb.tile([128, FULLB + 1, C], f32)
            nc.vector.tensor_copy(dist[:].rearrange("p b c -> p (b c)"), trd[:])
            rsum = sb.tile([128, FULLB + 1], f32)
            nc.vector.tensor_reduce(
                rsum[:], dist[:], axis=mybir.AxisListType.X,
                op=mybir.AluOpType.add,
            )
            rinv = sb.tile([128, FULLB + 1], f32)
            # valid rows always have rsum >= 1 (self-loop); clamp the zeroed
            # tail-garbage rows so 1/rsum stays finite (their dist is 0)
            nc.vector.tensor_scalar_max(rsum[:], rsum[:], 0.5)
            nc.vector.reciprocal(rinv[:], rsum[:])
            dd = sb.tile([128, FULLB + 1, C], f32)
            nc.vector.tensor_tensor(
                dd[:], dist[:],
                rinv[:].unsqueeze(2).broadcast_to([128, FULLB + 1, C]),
                mybir.AluOpType.mult,
            )
            nc.vector.tensor_tensor(dd[:], dd[:], pst[:], mybir.AluOpType.subtract)
            nc.vector.tensor_tensor(dd[:], dd[:], dd[:], mybir.AluOpType.mult)
            osq = sb.tile([128, FULLB + 1], f32)
            nc.vector.tensor_reduce(
                osq[:], dd[:], axis=mybir.AxisListType.X, op=mybir.AluOpType.add,
            )
            nc.sync.dma_start(out=out_d[:], in_=osq[:])

    nc.compile()
    return nc


_nc = None


def _get_program():
    global _nc
    if _nc is None:
        _install_neff_cache()
        _nc = build_program()
    return _nc


def prep_inputs(adj, labels_onehot, pseudo_labels, gumbel, train_mask):
    adj = np.asarray(adj, np.float32)
    labels = np.asarray(labels_onehot, np.float32)
    pseudo = np.asarray(pseudo_labels, np.float32)
    gumbel = np.asarray(gumbel, np.float32)
    m = np.asarray(train_mask).astype(bool)

    def tile_full(x, dtype):
        """[N,cols] -> [128, 79*cols] chunk-tiled, zero-padded."""
        cols = x.shape[1]
        p = np.zeros((NCHT * 128, cols), x.dtype)
        p[:N] = x
        return np.ascontiguousarray(
            p.reshape(NCHT, 128, cols).transpose(1, 0, 2).reshape(128, NCHT * cols)
        ).astype(dtype)

    labm = labels * m[:, None]
    gumt = tile_full(gumbel, np.float32)
    labmt = tile_full(labm, np.float16)
    m16 = np.repeat(m[:, None].astype(np.uint8), C, axis=1)
    m16t = tile_full(m16, np.uint8)

    in_maps = []
    for c in range(NCORES):
        rows = slice(c * R, (c + 1) * R)
        blk = np.ascontiguousarray(adj[rows, :].T)          # [N, R]
        adjT8 = blk.astype(F8)
        maskT8 = (blk != 0).astype(F8)
        mT16 = np.ascontiguousarray(
            np.broadcast_to(m[rows].astype(np.uint8), (C, R)))
        labT = np.ascontiguousarray(labm[rows].T.astype(np.float16))
        ps_loc = np.zeros(((FULLB + 1) * 128, C), np.float32)
        ps_loc[:R] = pseudo[rows]
        pst = np.ascontiguousarray(
            ps_loc.reshape(FULLB + 1, 128, C).transpose(1, 0, 2))
        in_maps.append({
            "adjT8": adjT8, "maskT8": maskT8, "gumt": gumt,
            "labmt": labmt, "m16t": m16t, "mT16": mT16, "labT": labT,
            "pst": pst,
        })
    return in_maps


def run_on_device(in_maps, trace=False, **kw):
    nc = _get_program()
    return run_bass_kernel_spmd(nc, in_maps, list(range(NCORES)), trace=trace, **kw)


def kernel(adj, labels_onehot, pseudo_labels, gumbel, train_mask,
           iter_step=10, k_hop=1, **_unused):
    assert int(iter_step) == 10 and int(k_hop) == 1, "kernel hardcodes 10/1"
    in_maps = prep_inputs(adj, labels_onehot, pseudo_labels, gumbel, train_mask)
    res = run_on_device(in_maps)
    total = 0.0
    for c in range(NCORES):
        sq = np.asarray(res.results[c]["out_sq"], np.float64)
        total += sq.sum()
    return np.float32(total / (N * C))



# revision 5
# speedup vs baseline: 1.7364x; 1.7364x over previous
"""Trainium2 Bass kernel for nn_ContextLabel (GNN label propagation), v2.

Computation: 10 iterations of Y = masked(adj @ Y) on [10000,16], then
straight-through gumbel one-hot, dist = (adj!=0) @ Yh row-normalized,
output mean((dist - pseudo_labels)^2)  (scalar).

v2 strategy (vs v1 baseline at ~890us):
 - N padded to 10240 so each core owns R=1280=10*128 rows: no tail
   chunks anywhere, and the AllGather wire format is chunk-tiled.
 - Y kept in fp8 e4m3 (host-validated: relerr 1.6e-5, 3 argmax flips)
   so every propagation matmul runs perf_mode=DoubleRow: two 128-row
   contraction chunks per matmul = 2x moving-operand stream rate.
 - contraction chunks stored group-ordered (g0=local blocks 0-3,
   g1=4-7, g2=8-9 across all cores) and each iteration is emitted
   slice-by-slice: slice s's AllGather overlaps slices s+1/s+2 matmuls
   and the next iteration's group-g matmuls only wait for AG(t,g).
   This also removes the big PE idle gaps that kept HAM at 1.2GHz.
 - mask (adj!=0, fp8) prefetched into SBUF across iterations 2..8 on
   the sync queue; the last 36 chunks stream during the final pass.
"""

import hashlib
import os
import shutil
import sys
from pathlib import Path

import numpy as np
import ml_dtypes

sys.path.insert(0, "/opt/trn_rl_repo")

import concourse.bass as bass  # noqa: E402
import concourse.mybir as mybir  # noqa: E402
import concourse.tile as tile  # noqa: E402
from concourse import bacc  # noqa: E402
import concourse.bass2jax as bass2jax  # noqa: E402
from concourse.bass_utils import run_bass_kernel_spmd  # noqa: E402
from concourse.masks import make_identity  # noqa: E402

F8 = ml_dtypes.float8_e4m3
NCORES = 8
N = 10000
NP = 10240                 # padded
C = 16
R = NP // NCORES           # 1280 rows per core
NB = R // 128              # 10 local blocks
CH = NP // 128             # 80 contraction chunks
GSIZE = [32, 32, 16]       # chunks per group (blocks 0-3 / 4-7 / 8-9)
GOFF = [0, 32, 64]         # group chunk-position offsets
SLICES = [(0, 512, 0, 4), (512, 512, 4, 4), (1024, 256, 8, 2)]  # (col0,w,b0,nb)
NRES = 44                  # mask chunks prefetched resident
NSTRM = CH - NRES          # 36 -> 18 streamed pairs
DR = mybir.MatmulPerfMode.DoubleRow

_NEFF_CACHE = Path.home() / ".cache" / "bass_neff"


def _install_neff_cache():
    orig = bass2jax.compile_bir_kernel
    if getattr(bass2jax.compile_bir_kernel, "_cached", False):
        return

    def cached(bir_json, tmpdir, neff_name="file.neff"):
        h = hashlib.sha256(bir_json).hexdigest()
        p = _NEFF_CACHE / f"{h}.neff"
        dst = os.path.join(tmpdir, neff_name)
        if p.exists():
            shutil.copy(p, dst)
            return dst
        out = orig(bir_json, tmpdir, neff_name)
        try:
            _NEFF_CACHE.mkdir(parents=True, exist_ok=True)
            shutil.copy(out, p)
        except OSError:
            pass
        return out

    cached._cached = True
    bass2jax.compile_bir_kernel = cached


def build_program():
    nc = bacc.Bacc(
        "TRN2", target_bir_lowering=False, debug=False,
        enable_asserts=False, num_devices=NCORES,
    )
    f8, f16, f32 = mybir.dt.float8e4, mybir.dt.float16, mybir.dt.float32
    u8 = mybir.dt.uint8

    adjT_d = nc.dram_tensor("adjT8", [NP, R], f8, kind="ExternalInput")
    maskT_d = nc.dram_tensor("maskT8", [NP, R], f8, kind="ExternalInput")
    labm8t_d = nc.dram_tensor("labm8t", [128, CH * C], f8, kind="ExternalInput")
    gumt_d = nc.dram_tensor("gumt", [128, CH * C], f32, kind="ExternalInput")
    m16t_d = nc.dram_tensor("m16t", [128, CH * C], u8, kind="ExternalInput")
    m16loc_d = nc.dram_tensor("m16loc", [128, NB * C], u8, kind="ExternalInput")
    labm8loc_d = nc.dram_tensor("labm8loc", [128, NB * C], f8, kind="ExternalInput")
    pst_d = nc.dram_tensor("pst", [128, NB * C], f32, kind="ExternalInput")
    out_d = nc.dram_tensor("out_sq", [128, NB], f32, kind="ExternalOutput")

    with tile.TileContext(nc) as tc:
        with (
            tc.tile_pool(name="sb", bufs=1) as sb,
            tc.tile_pool(name="yc", bufs=2) as yc,
            tc.tile_pool(name="pp", bufs=2) as pp,
            tc.tile_pool(name="mst", bufs=4) as mst,
            tc.tile_pool(name="ps", bufs=2, space="PSUM") as ps,
            tc.tile_pool(name="dram", bufs=2, space="DRAM") as dram,
        ):
            # ---- resident tiles -------------------------------------
            at_g = [sb.tile([128, GSIZE[g] * R], f8, name=f"at{g}") for g in range(3)]
            mask_res = sb.tile([128, NRES * R], f8)
            gumt = sb.tile([128, CH * C], f32)
            m16t = sb.tile([128, CH * C], u8)
            labm8t = sb.tile([128, CH * C], f8)
            m16loc = sb.tile([128, NB, C], u8)
            labm8loc = sb.tile([128, NB, C], f8)
            pst = sb.tile([128, NB, C], f32)
            ident16 = sb.tile([C, C], f16)
            ident32 = sb.tile([C, C], f32)
            rmax = sb.tile([128, CH], f32)
            yh8 = [sb.tile([128, GSIZE[g] * C], f8, name=f"yh{g}") for g in range(3)]

            make_identity(nc, ident16[:])
            make_identity(nc, ident32[:])

            # ycur double-buffered per group: iter t reads slot t%2
            ycur = [[yc.tile([128, GSIZE[g] * C], f8, name=f"yc{g}", tag=f"yc{g}")
                     for g in range(3)] for _ in range(2)]

            # ---- initial small loads (before adjT so iter1 can start)
            for g in range(3):
                nc.sync.dma_start(
                    out=ycur[1][g][:],
                    in_=labm8t_d[:, GOFF[g] * C:(GOFF[g] + GSIZE[g]) * C])
            nc.sync.dma_start(out=gumt[:], in_=gumt_d[:])
            nc.sync.dma_start(out=m16t[:], in_=m16t_d[:])
            nc.sync.dma_start(out=labm8t[:], in_=labm8t_d[:])
            nc.sync.dma_start(out=m16loc[:].rearrange("p b c -> p (b c)"),
                              in_=m16loc_d[:])
            nc.sync.dma_start(out=labm8loc[:].rearrange("p b c -> p (b c)"),
                              in_=labm8loc_d[:])
            nc.sync.dma_start(out=pst[:].rearrange("p b c -> p (b c)"), in_=pst_d[:])

            # ---- adjT group loads -----------------------------------
            for g in range(3):
                r0 = GOFF[g] * 128
                nc.sync.dma_start(
                    out=at_g[g][:].rearrange("p (k i) -> p k i", k=GSIZE[g]),
                    in_=adjT_d[r0:r0 + GSIZE[g] * 128, :]
                        .rearrange("(k p) i -> p k i", p=128),
                )

            # mask prefetch DMA schedule: list of (chunk0, nchunks),
            # one issued into the sync queue per slot during iters 2..8
            mask_sched = [(0, 6), (6, 6), (12, 6), (18, 6), (24, 7), (31, 7), (38, 6)]
            mask_it = iter(mask_sched)

            def issue_mask_prefetch():
                nx = next(mask_it, None)
                if nx is None:
                    return
                k0, nk = nx
                nc.sync.dma_start(
                    out=mask_res[:, k0 * R:(k0 + nk) * R]
                        .rearrange("p (k i) -> p k i", k=nk),
                    in_=maskT_d[k0 * 128:(k0 + nk) * 128, :]
                        .rearrange("(k p) i -> p k i", p=128),
                )

            pairs_all = []
            for g in range(3):
                pairs_all += [(g, j) for j in range(GSIZE[g] // 2)]
            npair = len(pairs_all)

            def one_mm(acc, slot, pi, s, first_pi, last_pi):
                g, j = pairs_all[pi]
                c0, w, _, _ = SLICES[s]
                lhsT = slot[g][:].rearrange("p (k c) -> p k c", c=C)[
                    :, 2 * j:2 * j + 2, :]
                rhs = at_g[g][:].rearrange("p (k i) -> p k i", k=GSIZE[g])[
                    :, 2 * j:2 * j + 2, c0:c0 + w]
                nc.tensor.matmul(acc[:, c0:c0 + w], lhsT, rhs,
                                 start=pi == first_pi, stop=pi == last_pi,
                                 perf_mode=DR)

            def post_slice(t, s, acc, next_mm):
                """After slice s's MMs: copy->transpose->mask->DMA->AG->DMA-in.

                next_mm: callable emitting the next slice's first pair-MM
                (or None); called between the DVE copy and the transposes so
                the PE keeps streaming while the copy runs.
                """
                c0, w, b0, nb = SLICES[s]
                yT = pp.tile([C, w], f16, name=f"yT{t}_{s}", tag=f"yT{s}")
                nc.vector.tensor_copy(yT[:], acc[:, c0:c0 + w])
                if next_mm is not None:
                    next_mm()
                trp = ps.tile([128, 512], f16, name=f"trp{t}_{s}", tag="trp")
                for b in range(nb):
                    nc.tensor.transpose(
                        trp[:, b * C:(b + 1) * C],
                        yT[:, b * 128:(b + 1) * 128], ident16[:])
                yloc = pp.tile([128, nb, C], f8, name=f"yl{t}_{s}", tag=f"yl{s}")
                nc.vector.tensor_copy(
                    yloc[:].rearrange("p b c -> p (b c)"), trp[:, 0:nb * C])
                nc.vector.copy_predicated(
                    yloc[:], m16loc[:, b0:b0 + nb, :], labm8loc[:, b0:b0 + nb, :])
                cc_in = dram.tile([128, nb * C], f8, name=f"ci{t}_{s}", tag=f"ci{s}")
                cc_out = dram.tile([128 * NCORES, nb * C], f8,
                                   name=f"co{t}_{s}", tag=f"co{s}",
                                   addr_space="Shared")
                nc.sync.dma_start(out=cc_in[:],
                                  in_=yloc[:].rearrange("p b c -> p (b c)"))
                nc.gpsimd.collective_compute(
                    "AllGather", mybir.AluOpType.bypass,
                    replica_groups=[list(range(NCORES))],
                    ins=[cc_in[:]], outs=[cc_out[:]],
                )
                # gather result -> ycur slot for iter t+1, group s
                dst = ycur[(t + 1) % 2][s]
                nc.sync.dma_start(
                    out=dst[:].rearrange("p (c f) -> p c f", c=NCORES),
                    in_=cc_out[:].rearrange("(c p) f -> p c f", p=128),
                )

            # ---- iteration 1: pair-outer, tracks adjT DMA ----------
            acc = ps.tile([C, 1536], f32, name="acc1", tag="acc")
            for pi in range(npair):
                for s in range(3):
                    one_mm(acc, ycur[1], pi, s, 0, npair - 1)
            # iteration-1 post for all 3 slices (no next_mm interleave --
            # iter-2 slice-0 MMs are gated on AG(1,0) anyway)
            for s in range(3):
                post_slice(1, s, acc, None)

            # ---- iterations 2..10: slice-outer ----------------------
            for t in range(2, 11):
                acc = ps.tile([C, 1536], f32, name=f"acc{t}", tag="acc")
                issue_mask_prefetch()
                slot = ycur[t % 2]

                def first_mm(s, acc=acc, slot=slot):
                    return lambda: one_mm(acc, slot, 0, s, 0, npair - 1)

                for pi in range(npair):
                    one_mm(acc, slot, pi, 0, 0, npair - 1)
                post_slice(t, 0, acc, next_mm=first_mm(1))
                for pi in range(1, npair):
                    one_mm(acc, slot, pi, 1, 0, npair - 1)
                post_slice(t, 1, acc, next_mm=first_mm(2))
                for pi in range(1, npair):
                    one_mm(acc, slot, pi, 2, 0, npair - 1)
                post_slice(t, 2, acc, None)

            # ---- straight-through gumbel one-hot, per group ---------
            gum3 = gumt[:].rearrange("p (k c) -> p k c", c=C)
            m163 = m16t[:].rearrange("p (k c) -> p k c", c=C)
            lab3 = labm8t[:].rearrange("p (k c) -> p k c", c=C)
            yfin = ycur[11 % 2]
            for g in range(3):
                g0, gs = GOFF[g], GSIZE[g]
                nc.vector.tensor_tensor(
                    gumt[:, g0 * C:(g0 + gs) * C], yfin[g][:],
                    gumt[:, g0 * C:(g0 + gs) * C], mybir.AluOpType.add)
                nc.vector.tensor_reduce(
                    rmax[:, g0:g0 + gs], gum3[:, g0:g0 + gs, :],
                    axis=mybir.AxisListType.X, op=mybir.AluOpType.max)
                nc.vector.tensor_tensor(
                    yh8[g][:].rearrange("p (k c) -> p k c", c=C),
                    gum3[:, g0:g0 + gs, :],
                    rmax[:, g0:g0 + gs].unsqueeze(2).broadcast_to([128, gs, C]),
                    mybir.AluOpType.is_equal)
                nc.vector.copy_predicated(
                    yh8[g][:].rearrange("p (k c) -> p k c", c=C),
                    m163[:, g0:g0 + gs, :], lab3[:, g0:g0 + gs, :])

            # ---- final pass: dist^T = Yh^T @ maskT ------------------
            dacc = ps.tile([C, 1536], f32, name="dacc", tag="acc")
            strm_tiles = []
            for jj in range(NSTRM // 2):
                mt = mst.tile([128, 2 * R], f8, name=f"ms{jj}", tag="ms")
                k0 = NRES + 2 * jj
                nc.scalar.dma_start(
                    out=mt[:].rearrange("p (k i) -> p k i", k=2),
                    in_=maskT_d[k0 * 128:(k0 + 2) * 128, :]
                        .rearrange("(k p) i -> p k i", p=128))
                strm_tiles.append(mt)

            npair = len(pairs_all)
            for pi, (g, j) in enumerate(pairs_all):
                kpos = GOFF[g] + 2 * j  # global chunk position of the pair
                if kpos < NRES:
                    rhs_base = mask_res[:].rearrange("p (k i) -> p k i", k=NRES)
                    rsl = rhs_base[:, kpos:kpos + 2, :]
                else:
                    mt = strm_tiles[(kpos - NRES) // 2]
                    rsl = mt[:].rearrange("p (k i) -> p k i", k=2)
                lhsT = yh8[g][:].rearrange("p (k c) -> p k c", c=C)[
                    :, 2 * j:2 * j + 2, :]
                for s in range(3):
                    c0, w, _, _ = SLICES[s]
                    nc.tensor.matmul(dacc[:, c0:c0 + w], lhsT, rsl[:, :, c0:c0 + w],
                                     start=pi == 0, stop=pi == npair - 1,
                                     perf_mode=DR)

            # ---- normalize + squared error --------------------------
            dT = sb.tile([C, R], f32)
            nc.vector.tensor_copy(dT[:], dacc[:, 0:R])
            trd = ps.tile([128, NB * C], f32, tag="trp")
            for b in range(NB):
                nc.tensor.transpose(
                    trd[:, b * C:(b + 1) * C],
                    dT[:, b * 128:(b + 1) * 128], ident32[:])
            dist = sb.tile([128, NB, C], f32)
            nc.vector.tensor_copy(dist[:].rearrange("p b c -> p (b c)"), trd[:])
            rsum = sb.tile([128, NB], f32)
            nc.vector.tensor_reduce(
                rsum[:], dist[:], axis=mybir.AxisListType.X,
                op=mybir.AluOpType.add)
            rinv = sb.tile([128, NB], f32)
            # pad rows (real row >= 10000) have dist==0 -> rsum 0; clamp so
            # 1/rsum stays finite (their dd is then 0-0=0)
            nc.vector.tensor_scalar_max(rsum[:], rsum[:], 0.5)
            nc.vector.reciprocal(rinv[:], rsum[:])
            dd = sb.tile([128, NB, C], f32)
            nc.vector.tensor_tensor(
                dd[:], dist[:],
                rinv[:].unsqueeze(2).broadcast_to([128, NB, C]),
                mybir.AluOpType.mult)
            nc.vector.tensor_tensor(dd[:], dd[:], pst[:], mybir.AluOpType.subtract)
            nc.vector.tensor_tensor(dd[:], dd[:], dd[:], mybir.AluOpType.mult)
            osq = sb.tile([128, NB], f32)
            nc.vector.tensor_reduce(
                osq[:], dd[:], axis=mybir.AxisListType.X, op=mybir.AluOpType.add)
            nc.sync.dma_start(out=out_d[:], in_=osq[:])

    nc.compile()
    return nc


_nc = None


def _get_program():
    global _nc
    if _nc is None:
        _install_neff_cache()
        _nc = build_program()
    return _nc


def _chunk_perm():
    """Global chunk position -> original chunk index k (group ordering)."""
    order = []
    for blo, bhi in ((0, 4), (4, 8), (8, 10)):
        for c in range(NCORES):
            for b in range(blo, bhi):
                order.append(NB * c + b)
    return np.array(order)


def prep_inputs(adj, labels_onehot, pseudo_labels, gumbel, train_mask):
    adj = np.asarray(adj, np.float32)
    labels = np.asarray(labels_onehot, np.float32)
    pseudo = np.asarray(pseudo_labels, np.float32)
    gumbel = np.asarray(gumbel, np.float32)
    m = np.asarray(train_mask).astype(bool)

    order = _chunk_perm()
    rowperm = (order[:, None] * 128 + np.arange(128)[None, :]).reshape(-1)

    labm = labels * m[:, None]

    def pad_rows(x, cols):
        p = np.zeros((NP, cols), x.dtype)
        p[:N] = x
        return p

    labm_p = pad_rows(labm, C)
    gum_p = pad_rows(gumbel, C)
    m16_p = pad_rows(np.repeat(m[:, None].astype(np.uint8), C, axis=1), C)
    pseudo_p = pad_rows(pseudo, C)

    def tile_global(xp, dtype):
        """[NP,C] -> [128, CH*C] in group-ordered chunk tiling."""
        t = xp[rowperm].reshape(CH, 128, C).transpose(1, 0, 2).reshape(128, CH * C)
        return np.ascontiguousarray(t).astype(dtype)

    labm8t = tile_global(labm_p, F8)
    gumt = tile_global(gum_p, np.float32)
    m16t = tile_global(m16_p, np.uint8)

    adj_p = np.zeros((NP, NP), np.float32)
    adj_p[:N, :N] = adj

    in_maps = []
    for c in range(NCORES):
        rows = slice(c * R, (c + 1) * R)
        blk = np.ascontiguousarray(adj_p[rows, :][:, rowperm].T)  # [NP, R]
        adjT8 = blk.astype(F8)
        maskT8 = (blk != 0).astype(F8)

        def tile_local(xp, dtype):
            t = xp[rows].reshape(NB, 128, C).transpose(1, 0, 2)
            return np.ascontiguousarray(t.reshape(128, NB * C)).astype(dtype)

        in_maps.append({
            "adjT8": adjT8, "maskT8": maskT8, "labm8t": labm8t,
            "gumt": gumt, "m16t": m16t,
            "m16loc": tile_local(m16_p, np.uint8),
            "labm8loc": tile_local(labm_p, F8),
            "pst": tile_local(pseudo_p, np.float32),
        })
    return in_maps


def run_on_device(in_maps, trace=False, **kw):
    nc = _get_program()
    return run_bass_kernel_spmd(nc, in_maps, list(range(NCORES)), trace=trace, **kw)


def kernel(adj, labels_onehot, pseudo_labels, gumbel, train_mask,
           iter_step=10, k_hop=1, **_unused):
    assert int(iter_step) == 10 and int(k_hop) == 1, "kernel hardcodes 10/1"
    in_maps = prep_inputs(adj, labels_onehot, pseudo_labels, gumbel, train_mask)
    res = run_on_device(in_maps)
    total = 0.0
    for c in range(NCORES):
        sq = np.asarray(res.results[c]["out_sq"], np.float64)
        total += sq.sum()
    return np.float32(total / (N * C))


# revision 8
# speedup vs baseline: 1.9500x; 1.1230x over previous
"""Trainium2 Bass kernel for nn_ContextLabel (GNN label propagation), v2.

Computation: 10 iterations of Y = masked(adj @ Y) on [10000,16], then
straight-through gumbel one-hot, dist = (adj!=0) @ Yh row-normalized,
output mean((dist - pseudo_labels)^2)  (scalar).

v2 strategy (vs v1 baseline at ~890us):
 - N padded to 10240 so each core owns R=1280=10*128 rows: no tail
   chunks anywhere, and the AllGather wire format is chunk-tiled.
 - Y kept in fp8 e4m3 (host-validated: relerr 1.6e-5, 3 argmax flips)
   so every propagation matmul runs perf_mode=DoubleRow: two 128-row
   contraction chunks per matmul = 2x moving-operand stream rate.
 - contraction chunks stored group-ordered (g0=local blocks 0-3,
   g1=4-7, g2=8-9 across all cores) and each iteration is emitted
   slice-by-slice: slice s's AllGather overlaps slices s+1/s+2 matmuls
   and the next iteration's group-g matmuls only wait for AG(t,g).
   This also removes the big PE idle gaps that kept HAM at 1.2GHz.
 - mask (adj!=0, fp8) prefetched into SBUF across iterations 2..8 on
   the sync queue; the last 36 chunks stream during the final pass.
"""

import hashlib
import os
import shutil
import sys
from pathlib import Path

import numpy as np
import ml_dtypes

sys.path.insert(0, "/opt/trn_rl_repo")

import concourse.bass as bass  # noqa: E402
import concourse.mybir as mybir  # noqa: E402
import concourse.tile as tile  # noqa: E402
from concourse import bacc  # noqa: E402
import concourse.bass2jax as bass2jax  # noqa: E402
from concourse.bass_utils import run_bass_kernel_spmd  # noqa: E402
from concourse.masks import make_identity  # noqa: E402

F8 = ml_dtypes.float8_e4m3
NCORES = 8
N = 10000
NP = 10240                 # padded
C = 16
R = NP // NCORES           # 1280 rows per core
NB = R // 128              # 10 local blocks
CH = NP // 128             # 80 contraction chunks
GSIZE = [32, 32, 16]       # chunks per group (blocks 0-3 / 4-7 / 8-9)
GOFF = [0, 32, 64]         # group chunk-position offsets
SLICES = [(0, 512, 0, 4), (512, 512, 4, 4), (1024, 256, 8, 2)]  # (col0,w,b0,nb)
NRES = 44                  # mask chunks prefetched resident
NSTRM = CH - NRES          # 36 -> 18 streamed pairs
DR = mybir.MatmulPerfMode.DoubleRow

_NEFF_CACHE = Path.home() / ".cache" / "bass_neff"


def _install_neff_cache():
    orig = bass2jax.compile_bir_kernel
    if getattr(bass2jax.compile_bir_kernel, "_cached", False):
        return

    def cached(bir_json, tmpdir, neff_name="file.neff"):
        h = hashlib.sha256(bir_json).hexdigest()
        p = _NEFF_CACHE / f"{h}.neff"
        dst = os.path.join(tmpdir, neff_name)
        if p.exists():
            shutil.copy(p, dst)
            return dst
        out = orig(bir_json, tmpdir, neff_name)
        try:
            _NEFF_CACHE.mkdir(parents=True, exist_ok=True)
            shutil.copy(out, p)
        except OSError:
            pass
        return out

    cached._cached = True
    bass2jax.compile_bir_kernel = cached


def build_program():
    nc = bacc.Bacc(
        "TRN2", target_bir_lowering=False, debug=False,
        enable_asserts=False, num_devices=NCORES,
    )
    f8, f16, f32 = mybir.dt.float8e4, mybir.dt.float16, mybir.dt.float32
    u8 = mybir.dt.uint8

    adjT_d = nc.dram_tensor("adjT8", [NP, R], f8, kind="ExternalInput")
    maskT_d = nc.dram_tensor("maskT8", [NP, R], f8, kind="ExternalInput")
    labm8t_d = nc.dram_tensor("labm8t", [128, CH * C], f8, kind="ExternalInput")
    gumt_d = nc.dram_tensor("gumt", [128, CH * C], f32, kind="ExternalInput")
    m16t_d = nc.dram_tensor("m16t", [128, CH * C], u8, kind="ExternalInput")
    m16loc_d = nc.dram_tensor("m16loc", [128, NB * C], u8, kind="ExternalInput")
    labm8loc_d = nc.dram_tensor("labm8loc", [128, NB * C], f8, kind="ExternalInput")
    pst_d = nc.dram_tensor("pst", [128, NB * C], f32, kind="ExternalInput")
    out_d = nc.dram_tensor("out_sq", [128, NB], f32, kind="ExternalOutput")

    with tile.TileContext(nc) as tc:
        with (
            tc.tile_pool(name="sb", bufs=1) as sb,
            tc.tile_pool(name="yc", bufs=2) as yc,
            tc.tile_pool(name="pp", bufs=2) as pp,
            tc.tile_pool(name="mst", bufs=4) as mst,
            tc.tile_pool(name="ps", bufs=2, space="PSUM") as ps,
            tc.tile_pool(name="dram", bufs=2, space="DRAM") as dram,
        ):
            # ---- resident tiles -------------------------------------
            at_g = [sb.tile([128, GSIZE[g] * R], f8, name=f"at{g}") for g in range(3)]
            mask_res = sb.tile([128, NRES * R], f8)
            gumt = sb.tile([128, CH * C], f32)
            m16t = sb.tile([128, CH * C], u8)
            labm8t = sb.tile([128, CH * C], f8)
            m16loc = sb.tile([128, NB, C], u8)
            labm8loc = sb.tile([128, NB, C], f8)
            pst = sb.tile([128, NB, C], f32)
            ident16 = sb.tile([C, C], f16)
            ident32 = sb.tile([C, C], f32)
            rmax = sb.tile([128, CH], f32)
            yh8 = [sb.tile([128, GSIZE[g] * C], f8, name=f"yh{g}") for g in range(3)]

            make_identity(nc, ident16[:])
            make_identity(nc, ident32[:])

            # ycur double-buffered per group: iter t reads slot t%2
            ycur = [[yc.tile([128, GSIZE[g] * C], f8, name=f"yc{g}", tag=f"yc{g}")
                     for g in range(3)] for _ in range(2)]

            # ---- initial small loads (before adjT so iter1 can start)
            for g in range(3):
                nc.sync.dma_start(
                    out=ycur[1][g][:],
                    in_=labm8t_d[:, GOFF[g] * C:(GOFF[g] + GSIZE[g]) * C])
            nc.sync.dma_start(out=gumt[:], in_=gumt_d[:])
            nc.sync.dma_start(out=m16t[:], in_=m16t_d[:])
            nc.sync.dma_start(out=labm8t[:], in_=labm8t_d[:])
            nc.sync.dma_start(out=m16loc[:].rearrange("p b c -> p (b c)"),
                              in_=m16loc_d[:])
            nc.sync.dma_start(out=labm8loc[:].rearrange("p b c -> p (b c)"),
                              in_=labm8loc_d[:])
            nc.sync.dma_start(out=pst[:].rearrange("p b c -> p (b c)"), in_=pst_d[:])

            # ---- adjT group loads -----------------------------------
            for g in range(3):
                r0 = GOFF[g] * 128
                nc.sync.dma_start(
                    out=at_g[g][:].rearrange("p (k i) -> p k i", k=GSIZE[g]),
                    in_=adjT_d[r0:r0 + GSIZE[g] * 128, :]
                        .rearrange("(k p) i -> p k i", p=128),
                )

            # mask prefetch DMA schedule: list of (chunk0, nchunks),
            # one issued into the sync queue per slot during iters 2..8
            mask_sched = [(0, 6), (6, 6), (12, 6), (18, 6), (24, 7), (31, 7), (38, 6)]
            mask_it = iter(mask_sched)

            def issue_mask_prefetch():
                nx = next(mask_it, None)
                if nx is None:
                    return
                k0, nk = nx
                nc.sync.dma_start(
                    out=mask_res[:, k0 * R:(k0 + nk) * R]
                        .rearrange("p (k i) -> p k i", k=nk),
                    in_=maskT_d[k0 * 128:(k0 + nk) * 128, :]
                        .rearrange("(k p) i -> p k i", p=128),
                )

            pairs_all = []
            for g in range(3):
                pairs_all += [(g, j) for j in range(GSIZE[g] // 2)]
            npair = len(pairs_all)
            NP0 = GSIZE[0] // 2            # 16 pairs in g0

            # pre-warm the collective path while adjT streams in (the
            # first AllGather otherwise pays ~30-50us of one-time setup)
            for wi in range(2):
                wa = dram.tile([128, 16], f8, name=f"wa{wi}", tag="wa")
                wb = dram.tile([128 * NCORES, 16], f8, name=f"wb{wi}", tag="wb",
                               addr_space="Shared")
                nc.gpsimd.collective_compute(
                    "AllGather", mybir.AluOpType.bypass,
                    replica_groups=[list(range(NCORES))],
                    ins=[wa[:]], outs=[wb[:]],
                )

            def one_mm(accs, slot, pi, s, first_pi, last_pi):
                g, j = pairs_all[pi]
                c0, w, _, _ = SLICES[s]
                lhsT = slot[g][:].rearrange("p (k c) -> p k c", c=C)[
                    :, 2 * j:2 * j + 2, :]
                rhs = at_g[g][:].rearrange("p (k i) -> p k i", k=GSIZE[g])[
                    :, 2 * j:2 * j + 2, c0:c0 + w]
                nc.tensor.matmul(accs[s][:, 0:w], lhsT, rhs,
                                 start=pi == first_pi, stop=pi == last_pi,
                                 perf_mode=DR)

            def post_slice(t, s, accs, next_mm, ag12=None):
                """After slice s's MMs: copy -> transpose -> mask -> exchange.

                s==0: own AllGather (cc0).  s==1: write into ag12 dram tile
                cols 0:64, no collective.  s==2: write ag12 cols 64:96, then
                one merged AllGather covering groups 1+2.
                next_mm emits the next MM block's first matmul between the
                DVE copy and the transposes to keep the PE streaming.
                """
                c0, w, b0, nb = SLICES[s]
                yT = pp.tile([C, w], f16, name=f"yT{t}_{s}", tag=f"yT{s}")
                nc.vector.tensor_copy(yT[:], accs[s][:, 0:w])
                if next_mm is not None:
                    next_mm()
                trp = ps.tile([128, 512], f16, name=f"trp{t}_{s}", tag="trp")
                for b in range(nb):
                    nc.tensor.transpose(
                        trp[:, b * C:(b + 1) * C],
                        yT[:, b * 128:(b + 1) * 128], ident16[:])
                yloc = pp.tile([128, nb, C], f8, name=f"yl{t}_{s}", tag=f"yl{s}")
                nc.vector.tensor_copy(
                    yloc[:].rearrange("p b c -> p (b c)"), trp[:, 0:nb * C])
                nc.vector.copy_predicated(
                    yloc[:], m16loc[:, b0:b0 + nb, :], labm8loc[:, b0:b0 + nb, :])
                if s == 0:
                    cc_in = dram.tile([128, 64], f8, name=f"ci0_{t}", tag="ci0")
                    cc_out = dram.tile([128 * NCORES, 64], f8,
                                       name=f"co0_{t}", tag="co0",
                                       addr_space="Shared")
                    nc.sync.dma_start(out=cc_in[:],
                                      in_=yloc[:].rearrange("p b c -> p (b c)"))
                    nc.gpsimd.collective_compute(
                        "AllGather", mybir.AluOpType.bypass,
                        replica_groups=[list(range(NCORES))],
                        ins=[cc_in[:]], outs=[cc_out[:]],
                    )
                    dst = ycur[(t + 1) % 2][0]
                    nc.scalar.dma_start(
                        out=dst[:].rearrange("p (c f) -> p c f", c=NCORES),
                        in_=cc_out[:].rearrange("(c p) f -> p c f", p=128),
                    )
                elif s == 1:
                    nc.sync.dma_start(out=ag12[0][:, 0:64],
                                      in_=yloc[:].rearrange("p b c -> p (b c)"))
                else:
                    cc_in, cc_out = ag12
                    nc.sync.dma_start(out=cc_in[:, 64:96],
                                      in_=yloc[:].rearrange("p b c -> p (b c)"))
                    nc.gpsimd.collective_compute(
                        "AllGather", mybir.AluOpType.bypass,
                        replica_groups=[list(range(NCORES))],
                        ins=[cc_in[:]], outs=[cc_out[:]],
                    )
                    for g, cols in ((1, (0, 64)), (2, (64, 96))):
                        dst = ycur[(t + 1) % 2][g]
                        nc.scalar.dma_start(
                            out=dst[:].rearrange("p (c f) -> p c f", c=NCORES),
                            in_=cc_out[:, cols[0]:cols[1]]
                                .rearrange("(c p) f -> p c f", p=128),
                        )

            def make_ag12(t):
                cc_in = dram.tile([128, 96], f8, name=f"ci12_{t}", tag="ci12")
                cc_out = dram.tile([128 * NCORES, 96], f8,
                                   name=f"co12_{t}", tag="co12",
                                   addr_space="Shared")
                return (cc_in, cc_out)

            def make_accs(t):
                return [ps.tile([C, 512], f32, name=f"a0_{t}", tag="a0"),
                        ps.tile([C, 512], f32, name=f"a1_{t}", tag="a1"),
                        ps.tile([C, 256], f32, name=f"a2_{t}", tag="a2")]

            # ---- iteration 1: pair-outer, tracks adjT DMA ----------
            accs = make_accs(1)
            ag12 = make_ag12(1)
            for pi in range(npair):
                for s in range(3):
                    one_mm(accs, ycur[1], pi, s, 0, npair - 1)
            for s in range(3):
                post_slice(1, s, accs, None, ag12=ag12)

            # ---- iterations 2..10 -----------------------------------
            # MM order: [s0,s1,s2: g0 pairs] then per-slice [g1,g2 pairs]
            # so the merged AG(g1+g2) of iter t-1 is only needed 8.5us in.
            for t in range(2, 11):
                accs = make_accs(t)
                ag12 = make_ag12(t)
                issue_mask_prefetch()
                slot = ycur[t % 2]

                def tail_mm(s, accs=accs, slot=slot):
                    return lambda: one_mm(accs, slot, NP0, s, 0, npair - 1)

                for s in range(3):
                    for pi in range(NP0):
                        one_mm(accs, slot, pi, s, 0, npair - 1)
                for pi in range(NP0, npair):
                    one_mm(accs, slot, pi, 0, 0, npair - 1)
                post_slice(t, 0, accs, next_mm=tail_mm(1), ag12=ag12)
                for pi in range(NP0 + 1, npair):
                    one_mm(accs, slot, pi, 1, 0, npair - 1)
                post_slice(t, 1, accs, next_mm=tail_mm(2), ag12=ag12)
                for pi in range(NP0 + 1, npair):
                    one_mm(accs, slot, pi, 2, 0, npair - 1)
                post_slice(t, 2, accs, None, ag12=ag12)

            # ---- straight-through gumbel one-hot, per group ---------
            gum3 = gumt[:].rearrange("p (k c) -> p k c", c=C)
            m163 = m16t[:].rearrange("p (k c) -> p k c", c=C)
            lab3 = labm8t[:].rearrange("p (k c) -> p k c", c=C)
            yfin = ycur[11 % 2]
            for g in range(3):
                g0, gs = GOFF[g], GSIZE[g]
                nc.vector.tensor_tensor(
                    gumt[:, g0 * C:(g0 + gs) * C], yfin[g][:],
                    gumt[:, g0 * C:(g0 + gs) * C], mybir.AluOpType.add)
                nc.vector.tensor_reduce(
                    rmax[:, g0:g0 + gs], gum3[:, g0:g0 + gs, :],
                    axis=mybir.AxisListType.X, op=mybir.AluOpType.max)
                nc.vector.tensor_tensor(
                    yh8[g][:].rearrange("p (k c) -> p k c", c=C),
                    gum3[:, g0:g0 + gs, :],
                    rmax[:, g0:g0 + gs].unsqueeze(2).broadcast_to([128, gs, C]),
                    mybir.AluOpType.is_equal)
                nc.vector.copy_predicated(
                    yh8[g][:].rearrange("p (k c) -> p k c", c=C),
                    m163[:, g0:g0 + gs, :], lab3[:, g0:g0 + gs, :])

            # ---- final pass: dist^T = Yh^T @ maskT ------------------
            dacc = make_accs(11)
            strm_tiles = []
            for jj in range(NSTRM // 2):
                mt = mst.tile([128, 2 * R], f8, name=f"ms{jj}", tag="ms")
                k0 = NRES + 2 * jj
                nc.scalar.dma_start(
                    out=mt[:].rearrange("p (k i) -> p k i", k=2),
                    in_=maskT_d[k0 * 128:(k0 + 2) * 128, :]
                        .rearrange("(k p) i -> p k i", p=128))
                strm_tiles.append(mt)

            npair = len(pairs_all)
            for pi, (g, j) in enumerate(pairs_all):
                kpos = GOFF[g] + 2 * j  # global chunk position of the pair
                if kpos < NRES:
                    rhs_base = mask_res[:].rearrange("p (k i) -> p k i", k=NRES)
                    rsl = rhs_base[:, kpos:kpos + 2, :]
                else:
                    mt = strm_tiles[(kpos - NRES) // 2]
                    rsl = mt[:].rearrange("p (k i) -> p k i", k=2)
                lhsT = yh8[g][:].rearrange("p (k c) -> p k c", c=C)[
                    :, 2 * j:2 * j + 2, :]
                for s in range(3):
                    c0, w, _, _ = SLICES[s]
                    nc.tensor.matmul(dacc[s][:, 0:w], lhsT, rsl[:, :, c0:c0 + w],
                                     start=pi == 0, stop=pi == npair - 1,
                                     perf_mode=DR)

            # ---- normalize + squared error --------------------------
            dT = sb.tile([C, R], f32)
            for s in range(3):
                c0, w, _, _ = SLICES[s]
                nc.vector.tensor_copy(dT[:, c0:c0 + w], dacc[s][:, 0:w])
            trd = ps.tile([128, NB * C], f32, tag="trp")
            for b in range(NB):
                nc.tensor.transpose(
                    trd[:, b * C:(b + 1) * C],
                    dT[:, b * 128:(b + 1) * 128], ident32[:])
            dist = sb.tile([128, NB, C], f32)
            nc.vector.tensor_copy(dist[:].rearrange("p b c -> p (b c)"), trd[:])
            rsum = sb.tile([128, NB], f32)
            nc.vector.tensor_reduce(
                rsum[:], dist[:], axis=mybir.AxisListType.X,
                op=mybir.AluOpType.add)
            rinv = sb.tile([128, NB], f32)
            # pad rows (real row >= 10000) have dist==0 -> rsum 0; clamp so
            # 1/rsum stays finite (their dd is then 0-0=0)
            nc.vector.tensor_scalar_max(rsum[:], rsum[:], 0.5)
            nc.vector.reciprocal(rinv[:], rsum[:])
            dd = sb.tile([128, NB, C], f32)
            nc.vector.tensor_tensor(
                dd[:], dist[:],
                rinv[:].unsqueeze(2).broadcast_to([128, NB, C]),
                mybir.AluOpType.mult)
            nc.vector.tensor_tensor(dd[:], dd[:], pst[:], mybir.AluOpType.subtract)
            nc.vector.tensor_tensor(dd[:], dd[:], dd[:], mybir.AluOpType.mult)
            osq = sb.tile([128, NB], f32)
            nc.vector.tensor_reduce(
                osq[:], dd[:], axis=mybir.AxisListType.X, op=mybir.AluOpType.add)
            nc.sync.dma_start(out=out_d[:], in_=osq[:])

    nc.compile()
    return nc


_nc = None


def _get_program():
    global _nc
    if _nc is None:
        _install_neff_cache()
        _nc = build_program()
    return _nc


def _chunk_perm():
    """Global chunk position -> original chunk index k (group ordering)."""
    order = []
    for blo, bhi in ((0, 4), (4, 8), (8, 10)):
        for c in range(NCORES):
            for b in range(blo, bhi):
                order.append(NB * c + b)
    return np.array(order)


def prep_inputs(adj, labels_onehot, pseudo_labels, gumbel, train_mask):
    adj = np.asarray(adj, np.float32)
    labels = np.asarray(labels_onehot, np.float32)
    pseudo = np.asarray(pseudo_labels, np.float32)
    gumbel = np.asarray(gumbel, np.float32)
    m = np.asarray(train_mask).astype(bool)

    order = _chunk_perm()
    rowperm = (order[:, None] * 128 + np.arange(128)[None, :]).reshape(-1)

    labm = labels * m[:, None]

    def pad_rows(x, cols):
        p = np.zeros((NP, cols), x.dtype)
        p[:N] = x
        return p

    labm_p = pad_rows(labm, C)
    gum_p = pad_rows(gumbel, C)
    m16_p = pad_rows(np.repeat(m[:, None].astype(np.uint8), C, axis=1), C)
    pseudo_p = pad_rows(pseudo, C)

    def tile_global(xp, dtype):
        """[NP,C] -> [128, CH*C] in group-ordered chunk tiling."""
        t = xp[rowperm].reshape(CH, 128, C).transpose(1, 0, 2).reshape(128, CH * C)
        return np.ascontiguousarray(t).astype(dtype)

    labm8t = tile_global(labm_p, F8)
    gumt = tile_global(gum_p, np.float32)
    m16t = tile_global(m16_p, np.uint8)

    adj_p = np.zeros((NP, NP), np.float32)
    adj_p[:N, :N] = adj

    in_maps = []
    for c in range(NCORES):
        rows = slice(c * R, (c + 1) * R)
        blk = np.ascontiguousarray(adj_p[rows, :][:, rowperm].T)  # [NP, R]
        adjT8 = blk.astype(F8)
        maskT8 = (blk != 0).astype(F8)

        def tile_local(xp, dtype):
            t = xp[rows].reshape(NB, 128, C).transpose(1, 0, 2)
            return np.ascontiguousarray(t.reshape(128, NB * C)).astype(dtype)

        in_maps.append({
            "adjT8": adjT8, "maskT8": maskT8, "labm8t": labm8t,
            "gumt": gumt, "m16t": m16t,
            "m16loc": tile_local(m16_p, np.uint8),
            "labm8loc": tile_local(labm_p, F8),
            "pst": tile_local(pseudo_p, np.float32),
        })
    return in_maps


def run_on_device(in_maps, trace=False, **kw):
    nc = _get_program()
    return run_bass_kernel_spmd(nc, in_maps, list(range(NCORES)), trace=trace, **kw)


def kernel(adj, labels_onehot, pseudo_labels, gumbel, train_mask,
           iter_step=10, k_hop=1, **_unused):
    assert int(iter_step) == 10 and int(k_hop) == 1, "kernel hardcodes 10/1"
    in_maps = prep_inputs(adj, labels_onehot, pseudo_labels, gumbel, train_mask)
    res = run_on_device(in_maps)
    total = 0.0
    for c in range(NCORES):
        sq = np.asarray(res.results[c]["out_sq"], np.float64)
        total += sq.sum()
    return np.float32(total / (N * C))
